# revision 5
# baseline (speedup 1.0000x reference)
"""Causal self-attention with RoPE on 8 Trainium2 NeuronCores.

Full inputs: x [4, 2048, 1024], W_attn [1024, 3072], W_proj [1024, 1024] (f32).
Sharding: core = b*2 + hg  (b in 0..3 batches, hg in 0..1 head-groups of 8 heads).
Each core computes qkv for its 8 heads, attention, and a partial output
projection (row-parallel c_proj); host sums the two partials per batch.

All matmuls run in float32r (fp32 with 11-bit mantissa, full PE rate at N>=256).
Matmul operands are pre-rounded (host inputs) or produced by DVE/ACT ops that
write float32r directly.
"""

import sys

sys.path.insert(0, "/opt/trn_rl_repo")

import numpy as np

import concourse.bass as bass  # noqa: F401
import concourse.mybir as mybir
import concourse.tile as tile
from concourse import bacc
from concourse.bass_utils import run_bass_kernel_spmd

F32 = mybir.dt.float32
F32R = mybir.dt.float32r
AF = mybir.ActivationFunctionType
OP = mybir.AluOpType

B, T, C = 4, 2048, 1024
H, D = 16, 64
HPC = 8            # heads per core
CO_QKV = 3 * HPC * D   # 1536 qkv columns per core
NEG = -30000.0     # additive mask; exp(S + NEG) == 0 exactly on ACT

TC = 256           # t-chunk width in phase A
N_TC = T // TC     # 8
N_KO = C // 128    # 8 contraction chunks
N_CT = 2 * HPC * D // 128   # 8 q+k column tiles (4 q, 4 k)
N_TT = T // 128    # 16 t tiles
N_IC = 4           # i-chunks of 512 queries
IC = 512


def round_fp32r(x):
    b = np.ascontiguousarray(x, dtype=np.float32).view(np.uint32)
    r = ((b.astype(np.uint64) + 0x800) & 0xFFFFF000).astype(np.uint32)
    return r.view(np.float32).reshape(x.shape)


def _rope_tables():
    """cosT/sinN [128, T] f32: row p holds freq for d = p % 64; sinN has the
    rotate-half sign folded in (rows d<32 negative)."""
    inv_freq = (
        np.float32(1.0)
        / np.float32(10000.0) ** (np.arange(0, D, 2, dtype=np.float32) / np.float32(D))
    ).astype(np.float32)
    t = np.arange(T, dtype=np.float32)
    freqs = (t[:, None] * inv_freq[None, :]).astype(np.float32)  # [T, 32]
    emb = np.concatenate([freqs, freqs], axis=1)  # [T, 64]
    cos = np.cos(emb).astype(np.float32)
    sin = np.sin(emb).astype(np.float32)
    sinN = np.concatenate([-sin[:, :32], sin[:, 32:]], axis=1)
    cosT = np.tile(cos.T, (2, 1))   # [128, T]
    sinNT = np.tile(sinN.T, (2, 1))
    return np.ascontiguousarray(cosT), np.ascontiguousarray(sinNT)


def _mask_table():
    """mask [128, 896]: mask[j, c] = 0 if j <= c - 384 else NEG."""
    j = np.arange(128)[:, None]
    c = np.arange(896)[None, :]
    return np.where(j <= c - 384, 0.0, NEG).astype(np.float32)


def _build():
    nc = bacc.Bacc(None, target_bir_lowering=False, debug=False)

    xT = nc.dram_tensor("xT", [C, T], F32R, kind="ExternalInput")
    wqkv = nc.dram_tensor("wqkv", [C, CO_QKV], F32R, kind="ExternalInput")
    wproj = nc.dram_tensor("wproj", [HPC * D, C], F32R, kind="ExternalInput")
    cosT_d = nc.dram_tensor("cosT", [128, T], F32, kind="ExternalInput")
    sinN_d = nc.dram_tensor("sinN", [128, T], F32, kind="ExternalInput")
    mask_d = nc.dram_tensor("mask", [128, 896], F32, kind="ExternalInput")
    ones_d = nc.dram_tensor("ones", [128, 64], F32R, kind="ExternalInput")
    out_d = nc.dram_tensor("out", [T, C], F32, kind="ExternalOutput")

    xT_r = xT.rearrange("(ko p) t -> p ko t", p=128)
    wqkv_r = wqkv.rearrange("(ko p) c -> p ko c", p=128)
    wproj_r = wproj.rearrange("(ko p) c -> p ko c", p=128)

    with tile.TileContext(nc) as tc:
        with (
            tc.tile_pool(name="resident", bufs=1) as res,
            tc.tile_pool(name="qkv", bufs=1) as qkv_pool,
        ):
            # ---- resident tables + outputs of phase A ----
            cos_sb = res.tile([128, T], F32)
            sinN_sb = res.tile([128, T], F32)
            mask_sb = res.tile([128, 896], F32)
            ones_sb = res.tile([128, 64], F32R)
            nc.sync.dma_start(cos_sb[:], cosT_d[:])
            nc.sync.dma_start(sinN_sb[:], sinN_d[:])
            nc.sync.dma_start(mask_sb[:], mask_d[:])
            nc.sync.dma_start(ones_sb[:], ones_d[:])

            # q^T / k^T: [p = d within head-pair, hp, t]
            qT = qkv_pool.tile([128, HPC // 2, T], F32R)
            kT = qkv_pool.tile([128, HPC // 2, T], F32R)
            # v: [p = t%128, t//128, head, 65] with ones column at d=64
            v_sb = qkv_pool.tile([128, N_TT, HPC, D + 1], F32R)
            # ---------------- Phase A: QKV + RoPE ----------------
            with (
                tc.tile_pool(name="w", bufs=1) as wp,
                tc.tile_pool(name="xt", bufs=2) as xtp,
                tc.tile_pool(name="rope", bufs=3) as ropep,
                tc.tile_pool(name="ps_qk", bufs=3, space="PSUM") as ps_qk,
                tc.tile_pool(name="ps_v", bufs=2, space="PSUM") as ps_v,
            ):
                w_sb = wp.tile([128, N_KO, CO_QKV], F32R)
                nc.sync.dma_start(w_sb[:], wqkv_r[:])

                # ones columns of v (written once)
                for to in range(N_TT):
                    nc.vector.tensor_copy(v_sb[:, to, :, D], ones_sb[:, 0:HPC])

                for tc_i in range(N_TC):
                    ts_ = slice(tc_i * TC, (tc_i + 1) * TC)
                    xt_sb = xtp.tile([128, N_KO, TC], F32R)
                    nc.sync.dma_start(xt_sb[:], xT_r[:, :, ts_])

                    # q and k column tiles (ct 0-3 -> q head-pair, 4-7 -> k)
                    for ct in range(N_CT):
                        psum = ps_qk.tile([128, TC], F32)
                        for ko in range(N_KO):
                            nc.tensor.matmul(
                                psum[:],
                                w_sb[:, ko, ct * 128 : (ct + 1) * 128],
                                xt_sb[:, ko, :],
                                start=(ko == 0),
                                stop=(ko == N_KO - 1),
                            )
                        hp = ct % 4
                        dest = (qT if ct < 4 else kT)[:, hp, ts_]
                        # RoPE: dest = psum * cos + shift(psum) * sinN
                        rot = ropep.tile([128, TC], F32)
                        for blk in range(4):
                            src = (blk ^ 1) * 32
                            nc.vector.tensor_copy(
                                rot[blk * 32 : blk * 32 + 32, :],
                                psum[src : src + 32, :],
                            )
                        nc.vector.tensor_tensor(rot[:], rot[:], sinN_sb[:, ts_], OP.mult)
                        nc.vector.tensor_tensor(dest, psum[:], cos_sb[:, ts_], OP.mult)
                        nc.vector.tensor_tensor(dest, dest, rot[:], OP.add)

                    # v tiles: two 128-row t sub-tiles per chunk
                    for sub in range(TC // 128):
                        to = tc_i * (TC // 128) + sub
                        psv = ps_v.tile([128, HPC * D], F32)
                        for ko in range(N_KO):
                            nc.tensor.matmul(
                                psv[:],
                                xt_sb[:, ko, sub * 128 : sub * 128 + 128],
                                w_sb[:, ko, 2 * HPC * D : 3 * HPC * D],
                                start=(ko == 0),
                                stop=(ko == N_KO - 1),
                            )
                        nc.vector.tensor_copy(
                            v_sb[:, to, :, 0:D],
                            psv[:].rearrange("p (h d) -> p h d", d=D),
                        )

            # ---------------- Phases B+C pool (opens after phase A frees W/xT) ----
            with tc.tile_pool(name="yt", bufs=1) as ytp:
                # y^T: same layout as qT
                yT = ytp.tile([128, HPC // 2, T], F32R)
                _phase_bc(nc, tc, qT, kT, v_sb, yT, mask_sb, ones_sb, wproj_r, out_d)

    nc.compile()
    return nc


def _phase_bc(nc, tc, qT, kT, v_sb, yT, mask_sb, ones_sb, wproj_r, out_d):
    if True:
        if True:
            # ---------------- Phase B: attention ----------------
            with (
                tc.tile_pool(name="exp", bufs=6) as expp,
                tc.tile_pool(name="smask", bufs=3) as smp,
                tc.tile_pool(name="fin", bufs=4) as finp,
                tc.tile_pool(name="ps_s", bufs=3, space="PSUM") as ps_s,
                tc.tile_pool(name="ps_y", bufs=2, space="PSUM") as ps_y,
                tc.tile_pool(name="ps_bc", bufs=2, space="PSUM") as ps_bc,
            ):
                for hp in range(HPC // 2):
                    for ic in range(N_IC):
                        is_ = slice(ic * IC, (ic + 1) * IC)
                        n_jt = (ic + 1) * 4
                        ypsum = [
                            ps_y.tile([D + 1, IC], F32, name=f"ypsum{_hl}", tag="ypsum")
                            for _hl in range(2)
                        ]
                        for jt in range(n_jt):
                            for hl in range(2):
                                pb = hl * 64
                                h = 2 * hp + hl
                                sps = ps_s.tile([128, IC], F32)
                                nc.tensor.matmul(
                                    sps[:],
                                    kT[pb : pb + 64, hp, jt * 128 : (jt + 1) * 128],
                                    qT[pb : pb + 64, hp, is_],
                                    start=True,
                                    stop=True,
                                )
                                expT = expp.tile([128, IC], F32R)
                                if jt >= ic * 4:
                                    off = 384 - (jt - ic * 4) * 128
                                    sm = smp.tile([128, IC], F32)
                                    nc.vector.tensor_tensor(
                                        sm[:], sps[:], mask_sb[:, off : off + IC], OP.add
                                    )
                                    nc.scalar.activation(expT[:], sm[:], AF.Exp)
                                else:
                                    nc.scalar.activation(expT[:], sps[:], AF.Exp)
                                nc.tensor.matmul(
                                    ypsum[hl][:],
                                    v_sb[:, jt, h, :],
                                    expT[:],
                                    start=(jt == 0),
                                    stop=(jt == n_jt - 1),
                                )
                        for hl in range(2):
                            pb = hl * 64
                            recip = finp.tile([1, IC], F32R)
                            with nc.allow_low_precision(reason="softmax recip f32r"):
                                nc.vector.reciprocal(recip[:], ypsum[hl][D : D + 1, :])
                            bc = ps_bc.tile([D, IC], F32)
                            nc.tensor.matmul(
                                bc[:], ones_sb[0:1, 0:D], recip[:], start=True, stop=True
                            )
                            bc_sb = finp.tile([D, IC], F32, name="bc_sb", tag="bc_sb")
                            nc.vector.tensor_copy(bc_sb[:], bc[:])
                            nc.vector.tensor_tensor(
                                yT[pb : pb + 64, hp, is_],
                                ypsum[hl][0:D, :],
                                bc_sb[:],
                                OP.mult,
                            )

            # ---------------- Phase C: output projection ----------------
            with (
                tc.tile_pool(name="wp2", bufs=1) as wp2,
                tc.tile_pool(name="ostage", bufs=4) as osp,
                tc.tile_pool(name="ps_o", bufs=4, space="PSUM") as ps_o,
            ):
                wp_sb = wp2.tile([128, 4, C], F32R)
                nc.sync.dma_start(wp_sb[:], wproj_r[:])
                for tt in range(N_TT):
                    for cc in range(C // 512):
                        po = ps_o.tile([128, 512], F32)
                        for ko in range(4):
                            nc.tensor.matmul(
                                po[:],
                                yT[:, ko, tt * 128 : (tt + 1) * 128],
                                wp_sb[:, ko, cc * 512 : (cc + 1) * 512],
                                start=(ko == 0),
                                stop=(ko == 3),
                            )
                        ost = osp.tile([128, 512], F32)
                        nc.vector.tensor_copy(ost[:], po[:])
                        nc.sync.dma_start(
                            out_d[tt * 128 : (tt + 1) * 128, cc * 512 : (cc + 1) * 512],
                            ost[:],
                        )


_NC = None


def _get_nc():
    global _NC
    if _NC is None:
        _NC = _build()
    return _NC


def kernel(x, W_attn, W_proj):
    x = np.asarray(x, dtype=np.float32)
    W_attn = np.asarray(W_attn, dtype=np.float32)
    W_proj = np.asarray(W_proj, dtype=np.float32)
    nc = _get_nc()

    cosT, sinN = _rope_tables()
    mask = _mask_table()
    ones = np.ones((128, 64), dtype=np.float32)
    scale = np.float32(1.0 / np.sqrt(D))

    in_maps = []
    for core in range(8):
        b, hg = core // 2, core % 2
        cs = slice(hg * HPC * D, (hg + 1) * HPC * D)
        wq = W_attn[:, 0 * C:][:, cs] * scale
        wk = W_attn[:, 1 * C:][:, cs]
        wv = W_attn[:, 2 * C:][:, cs]
        in_maps.append(
            {
                "xT": round_fp32r(x[b].T),
                "wqkv": round_fp32r(np.concatenate([wq, wk, wv], axis=1)),
                "wproj": round_fp32r(W_proj[cs, :]),
                "cosT": cosT,
                "sinN": sinN,
                "mask": mask,
                "ones": ones,
            }
        )

    res = run_bass_kernel_spmd(nc, in_maps, core_ids=list(range(8)))
    out = np.empty((B, T, C), dtype=np.float32)
    for b in range(B):
        out[b] = res.results[2 * b]["out"] + res.results[2 * b + 1]["out"]
    return out


# revision 6
# speedup vs baseline: 1.5614x; 1.5614x over previous
"""Causal self-attention with RoPE on 8 Trainium2 NeuronCores.

Full inputs: x [4, 2048, 1024], W_attn [1024, 3072], W_proj [1024, 1024] (f32).
Sharding: core = b*2 + hg  (b in 0..3 batches, hg in 0..1 head-groups of 8 heads).
Each core computes qkv for its 8 heads, attention, and a partial output
projection (row-parallel c_proj); host sums the two partials per batch.

All matmuls run in float32r (fp32 with 11-bit mantissa, full PE rate at N>=256).
Matmul operands are pre-rounded (host inputs) or produced by DVE/ACT ops that
write float32r directly.
"""

import sys

sys.path.insert(0, "/opt/trn_rl_repo")

import numpy as np

import concourse.bass as bass  # noqa: F401
import concourse.mybir as mybir
import concourse.tile as tile
from concourse import bacc
from concourse.bass_utils import run_bass_kernel_spmd

F32 = mybir.dt.float32
F32R = mybir.dt.float32r
AF = mybir.ActivationFunctionType
OP = mybir.AluOpType

B, T, C = 4, 2048, 1024
H, D = 16, 64
HPC = 8            # heads per core
CO_QKV = 3 * HPC * D   # 1536 qkv columns per core
NEG = -30000.0     # additive mask; exp(S + NEG) == 0 exactly on ACT

TC = 256           # t-chunk width in phase A
N_TC = T // TC     # 8
N_KO = C // 128    # 8 contraction chunks
N_CT = 2 * HPC * D // 128   # 8 q+k column tiles (4 q, 4 k)
N_TT = T // 128    # 16 t tiles
N_IC = 4           # i-chunks of 512 queries
IC = 512


def round_fp32r(x):
    b = np.ascontiguousarray(x, dtype=np.float32).view(np.uint32)
    r = ((b.astype(np.uint64) + 0x800) & 0xFFFFF000).astype(np.uint32)
    return r.view(np.float32).reshape(x.shape)


def _rope_tables():
    """cosT/sinN [128, T] f32: row p holds freq for d = p % 64; sinN has the
    rotate-half sign folded in (rows d<32 negative)."""
    inv_freq = (
        np.float32(1.0)
        / np.float32(10000.0) ** (np.arange(0, D, 2, dtype=np.float32) / np.float32(D))
    ).astype(np.float32)
    t = np.arange(T, dtype=np.float32)
    freqs = (t[:, None] * inv_freq[None, :]).astype(np.float32)  # [T, 32]
    emb = np.concatenate([freqs, freqs], axis=1)  # [T, 64]
    cos = np.cos(emb).astype(np.float32)
    sin = np.sin(emb).astype(np.float32)
    sinN = np.concatenate([-sin[:, :32], sin[:, 32:]], axis=1)
    cosT = np.tile(cos.T, (2, 1))   # [128, T]
    sinNT = np.tile(sinN.T, (2, 1))
    return np.ascontiguousarray(cosT), np.ascontiguousarray(sinNT)


def _mask_table():
    """mask [128, 896]: mask[j, c] = 0 if j <= c - 384 else NEG."""
    j = np.arange(128)[:, None]
    c = np.arange(896)[None, :]
    return np.where(j <= c - 384, 0.0, NEG).astype(np.float32)


def _build():
    nc = bacc.Bacc(None, target_bir_lowering=False, debug=False)

    xT = nc.dram_tensor("xT", [C, T], F32R, kind="ExternalInput")
    wqkv = nc.dram_tensor("wqkv", [C, CO_QKV], F32R, kind="ExternalInput")
    wproj = nc.dram_tensor("wproj", [HPC * D, C], F32R, kind="ExternalInput")
    cosT_d = nc.dram_tensor("cosT", [128, T], F32, kind="ExternalInput")
    sinN_d = nc.dram_tensor("sinN", [128, T], F32, kind="ExternalInput")
    mask_d = nc.dram_tensor("mask", [128, 896], F32, kind="ExternalInput")
    ones_d = nc.dram_tensor("ones", [128, 64], F32R, kind="ExternalInput")
    out_d = nc.dram_tensor("out", [T, C], F32, kind="ExternalOutput")

    xT_r = xT.rearrange("(ko p) t -> p ko t", p=128)
    wqkv_r = wqkv.rearrange("(ko p) c -> p ko c", p=128)
    wproj_r = wproj.rearrange("(ko p) c -> p ko c", p=128)

    with tile.TileContext(nc) as tc:
        with (
            tc.tile_pool(name="resident", bufs=1) as res,
            tc.tile_pool(name="qkv", bufs=1) as qkv_pool,
        ):
            # ---- resident tables + outputs of phase A ----
            cos_sb = res.tile([128, T], F32)
            sinN_sb = res.tile([128, T], F32)
            mask_sb = res.tile([128, 896], F32)
            ones_sb = res.tile([128, 64], F32R)
            nc.sync.dma_start(cos_sb[:], cosT_d[:])
            nc.sync.dma_start(sinN_sb[:], sinN_d[:])
            nc.sync.dma_start(mask_sb[:], mask_d[:])
            nc.sync.dma_start(ones_sb[:], ones_d[:])

            # q^T / k^T: [p = d within head-pair, hp, t]
            qT = qkv_pool.tile([128, HPC // 2, T], F32R)
            kT = qkv_pool.tile([128, HPC // 2, T], F32R)
            # v: [p = t%128, t//128, head, 65] with ones column at d=64
            v_sb = qkv_pool.tile([128, N_TT, HPC, D + 1], F32R)
            # ---------------- Phase A: QKV + RoPE ----------------
            with (
                tc.tile_pool(name="w", bufs=1) as wp,
                tc.tile_pool(name="xt", bufs=2) as xtp,
                tc.tile_pool(name="rope", bufs=3) as ropep,
                tc.tile_pool(name="ps_qk", bufs=3, space="PSUM") as ps_qk,
                tc.tile_pool(name="ps_v", bufs=2, space="PSUM") as ps_v,
            ):
                w_sb = wp.tile([128, N_KO, CO_QKV], F32R)
                nc.sync.dma_start(w_sb[:], wqkv_r[:])

                # ones columns of v (written once)
                for to in range(N_TT):
                    nc.vector.tensor_copy(v_sb[:, to, :, D], ones_sb[:, 0:HPC])

                for tc_i in range(N_TC):
                    ts_ = slice(tc_i * TC, (tc_i + 1) * TC)
                    xt_sb = xtp.tile([128, N_KO, TC], F32R)
                    nc.sync.dma_start(xt_sb[:], xT_r[:, :, ts_])

                    # q and k column tiles (ct 0-3 -> q head-pair, 4-7 -> k)
                    for ct in range(N_CT):
                        psum = ps_qk.tile([128, TC], F32)
                        for ko in range(N_KO):
                            nc.tensor.matmul(
                                psum[:],
                                w_sb[:, ko, ct * 128 : (ct + 1) * 128],
                                xt_sb[:, ko, :],
                                start=(ko == 0),
                                stop=(ko == N_KO - 1),
                            )
                        hp = ct % 4
                        dest = (qT if ct < 4 else kT)[:, hp, ts_]
                        # RoPE: dest = psum * cos + shift(psum) * sinN
                        rot = ropep.tile([128, TC], F32)
                        for blk in range(4):
                            src = (blk ^ 1) * 32
                            nc.vector.tensor_copy(
                                rot[blk * 32 : blk * 32 + 32, :],
                                psum[src : src + 32, :],
                            )
                        nc.vector.tensor_tensor(rot[:], rot[:], sinN_sb[:, ts_], OP.mult)
                        nc.vector.tensor_tensor(dest, psum[:], cos_sb[:, ts_], OP.mult)
                        nc.vector.tensor_tensor(dest, dest, rot[:], OP.add)

                    # v tiles: two 128-row t sub-tiles per chunk
                    for sub in range(TC // 128):
                        to = tc_i * (TC // 128) + sub
                        psv = ps_v.tile([128, HPC * D], F32)
                        for ko in range(N_KO):
                            nc.tensor.matmul(
                                psv[:],
                                xt_sb[:, ko, sub * 128 : sub * 128 + 128],
                                w_sb[:, ko, 2 * HPC * D : 3 * HPC * D],
                                start=(ko == 0),
                                stop=(ko == N_KO - 1),
                            )
                        nc.vector.tensor_copy(
                            v_sb[:, to, :, 0:D],
                            psv[:].rearrange("p (h d) -> p h d", d=D),
                        )

            # ---------------- Phases B+C pool (opens after phase A frees W/xT) ----
            with tc.tile_pool(name="yt", bufs=1) as ytp:
                # y^T: same layout as qT
                yT = ytp.tile([128, HPC // 2, T], F32R)
                _phase_bc(nc, tc, qT, kT, v_sb, yT, mask_sb, ones_sb, wproj_r, out_d)

    nc.compile()
    return nc


def _phase_bc(nc, tc, qT, kT, v_sb, yT, mask_sb, ones_sb, wproj_r, out_d):
    if True:
        if True:
            # ---------------- Phase B: attention ----------------
            with (
                tc.tile_pool(name="exp", bufs=6) as expp,
                tc.tile_pool(name="smask", bufs=3) as smp,
                tc.tile_pool(name="fin", bufs=4) as finp,
                tc.tile_pool(name="ps_s", bufs=3, space="PSUM") as ps_s,
                tc.tile_pool(name="ps_y", bufs=2, space="PSUM") as ps_y,
                tc.tile_pool(name="ps_bc", bufs=2, space="PSUM") as ps_bc,
            ):
                for hp in range(HPC // 2):
                    for ic in range(N_IC):
                        is_ = slice(ic * IC, (ic + 1) * IC)
                        n_jt = (ic + 1) * 4
                        ypsum = [
                            ps_y.tile([D + 1, IC], F32, name=f"ypsum{_hl}", tag="ypsum")
                            for _hl in range(2)
                        ]
                        for jt in range(n_jt):
                            for hl in range(2):
                                pb = hl * 64
                                h = 2 * hp + hl
                                sps = ps_s.tile([128, IC], F32)
                                nc.tensor.matmul(
                                    sps[:],
                                    kT[pb : pb + 64, hp, jt * 128 : (jt + 1) * 128],
                                    qT[pb : pb + 64, hp, is_],
                                    start=True,
                                    stop=True,
                                )
                                expT = expp.tile([128, IC], F32R)
                                if jt >= ic * 4:
                                    off = 384 - (jt - ic * 4) * 128
                                    sm = smp.tile([128, IC], F32)
                                    nc.vector.tensor_tensor(
                                        sm[:], sps[:], mask_sb[:, off : off + IC], OP.add
                                    )
                                    nc.scalar.activation(expT[:], sm[:], AF.Exp)
                                else:
                                    nc.scalar.activation(expT[:], sps[:], AF.Exp)
                                nc.tensor.matmul(
                                    ypsum[hl][:],
                                    v_sb[:, jt, h, :],
                                    expT[:],
                                    start=(jt == 0),
                                    stop=(jt == n_jt - 1),
                                )
                        for hl in range(2):
                            pb = hl * 64
                            recip = finp.tile([1, IC], F32R)
                            with nc.allow_low_precision(reason="softmax recip f32r"):
                                nc.vector.reciprocal(recip[:], ypsum[hl][D : D + 1, :])
                            bc = ps_bc.tile([D, IC], F32)
                            nc.tensor.matmul(
                                bc[:], ones_sb[0:1, 0:D], recip[:], start=True, stop=True
                            )
                            bc_sb = finp.tile([D, IC], F32, name="bc_sb", tag="bc_sb")
                            nc.vector.tensor_copy(bc_sb[:], bc[:])
                            nc.vector.tensor_tensor(
                                yT[pb : pb + 64, hp, is_],
                                ypsum[hl][0:D, :],
                                bc_sb[:],
                                OP.mult,
                            )

            # ---------------- Phase C: output projection ----------------
            with (
                tc.tile_pool(name="wp2", bufs=1) as wp2,
                tc.tile_pool(name="ostage", bufs=4) as osp,
                tc.tile_pool(name="ps_o", bufs=4, space="PSUM") as ps_o,
            ):
                wp_sb = wp2.tile([128, 4, C], F32R)
                nc.sync.dma_start(wp_sb[:], wproj_r[:])
                for tt in range(N_TT):
                    for cc in range(C // 512):
                        po = ps_o.tile([128, 512], F32)
                        for ko in range(4):
                            nc.tensor.matmul(
                                po[:],
                                yT[:, ko, tt * 128 : (tt + 1) * 128],
                                wp_sb[:, ko, cc * 512 : (cc + 1) * 512],
                                start=(ko == 0),
                                stop=(ko == 3),
                            )
                        ost = osp.tile([128, 512], F32)
                        nc.vector.tensor_copy(ost[:], po[:])
                        nc.sync.dma_start(
                            out_d[tt * 128 : (tt + 1) * 128, cc * 512 : (cc + 1) * 512],
                            ost[:],
                        )


_NC = None
_STATE = None


def _get_nc():
    global _NC
    if _NC is None:
        _NC = _build()
    return _NC


def _get_state():
    """Build the bass module once and cache a persistent jitted executor plus
    device-resident static tables (rope/mask/ones are pure functions of the
    problem shape)."""
    global _STATE
    if _STATE is not None:
        return _STATE

    import jax
    import jax.numpy as jnp
    from jax.experimental.shard_map import shard_map
    from jax.sharding import Mesh, NamedSharding, PartitionSpec

    from concourse import bass2jax

    nc = _get_nc()
    bass2jax.install_neuronx_cc_hook()
    partition_name = nc.partition_id_tensor.name if nc.partition_id_tensor else None
    in_names, out_names, out_avals = [], [], []
    for alloc in nc.m.functions[0].allocations:
        if not isinstance(alloc, mybir.MemoryLocationSet):
            continue
        name = alloc.memorylocations[0].name
        if alloc.kind == "ExternalInput":
            if name != partition_name:
                in_names.append(name)
        elif alloc.kind == "ExternalOutput":
            out_names.append(name)
            out_avals.append(
                jax.core.ShapedArray(tuple(alloc.tensor_shape), mybir.dt.np(alloc.dtype))
            )
    n_params, n_outs = len(in_names), len(out_avals)
    all_names = list(in_names) + out_names
    if partition_name:
        all_names.append(partition_name)

    def _body(*args):
        operands = list(args)
        if partition_name:
            operands.append(bass2jax.partition_id_tensor())
        outs = bass2jax._bass_exec_p.bind(
            *operands,
            out_avals=tuple(out_avals),
            in_names=tuple(all_names),
            out_names=tuple(out_names),
            lowering_input_output_aliases=(),
            sim_require_finite=True,
            sim_require_nnan=True,
            nc=nc,
        )
        return tuple(outs)

    devices = jax.devices()[:8]
    mesh = Mesh(np.asarray(devices), ("core",))
    shd = NamedSharding(mesh, PartitionSpec("core"))
    donate = tuple(range(n_params, n_params + n_outs))
    sharded = jax.jit(
        shard_map(
            _body,
            mesh=mesh,
            in_specs=(PartitionSpec("core"),) * (n_params + n_outs),
            out_specs=(PartitionSpec("core"),) * n_outs,
            check_rep=False,
        ),
        donate_argnums=donate,
        keep_unused=True,
    )
    zeros_fn = jax.jit(
        lambda: tuple(
            jnp.zeros((8 * av.shape[0],) + av.shape[1:], av.dtype) for av in out_avals
        ),
        out_shardings=(shd,) * n_outs,
    )

    cosT, sinN = _rope_tables()
    mask = _mask_table()
    ones = np.ones((128, 64), dtype=np.float32)
    statics = {
        "cosT": jax.device_put(np.tile(cosT, (8, 1)), shd),
        "sinN": jax.device_put(np.tile(sinN, (8, 1)), shd),
        "mask": jax.device_put(np.tile(mask, (8, 1)), shd),
        "ones": jax.device_put(np.tile(ones, (8, 1)), shd),
    }
    jax.block_until_ready(list(statics.values()))

    _STATE = dict(
        jax=jax,
        nc=nc,
        in_names=in_names,
        out_names=out_names,
        n_outs=n_outs,
        sharded=sharded,
        zeros_fn=zeros_fn,
        shd=shd,
        statics=statics,
    )
    return _STATE


def _prep_inputs(x, W_attn, W_proj):
    """Assemble the per-core concatenated dynamic inputs (host side)."""
    scale = np.float32(1.0 / np.sqrt(D))
    xcat = np.empty((8 * C, T), dtype=np.float32)
    for b in range(B):
        xt = round_fp32r(x[b].T)
        xcat[(2 * b) * C : (2 * b + 1) * C] = xt
        xcat[(2 * b + 1) * C : (2 * b + 2) * C] = xt
    wqkvcat = np.empty((8 * C, CO_QKV), dtype=np.float32)
    wprojcat = np.empty((8 * HPC * D, C), dtype=np.float32)
    for hg in range(2):
        cs = slice(hg * HPC * D, (hg + 1) * HPC * D)
        wq = W_attn[:, 0 * C:][:, cs] * scale
        wk = W_attn[:, 1 * C:][:, cs]
        wv = W_attn[:, 2 * C:][:, cs]
        wqkv = round_fp32r(np.concatenate([wq, wk, wv], axis=1))
        wproj = round_fp32r(W_proj[cs, :])
        for b in range(B):
            core = 2 * b + hg
            wqkvcat[core * C : (core + 1) * C] = wqkv
            wprojcat[core * HPC * D : (core + 1) * HPC * D] = wproj
    return {"xT": xcat, "wqkv": wqkvcat, "wproj": wprojcat}


def kernel(x, W_attn, W_proj):
    x = np.asarray(x, dtype=np.float32)
    W_attn = np.asarray(W_attn, dtype=np.float32)
    W_proj = np.asarray(W_proj, dtype=np.float32)

    st = _get_state()
    jax = st["jax"]
    dyn = _prep_inputs(x, W_attn, W_proj)
    args = []
    for nm in st["in_names"]:
        if nm in dyn:
            args.append(jax.device_put(dyn[nm], st["shd"]))
        else:
            args.append(st["statics"][nm])
    zeros = st["zeros_fn"]()
    outs = st["sharded"](*args, *zeros)
    r = np.asarray(outs[0]).reshape(8, T, C)
    out = np.empty((B, T, C), dtype=np.float32)
    for b in range(B):
        np.add(r[2 * b], r[2 * b + 1], out=out[b])
    return out


# revision 9
# speedup vs baseline: 3.7498x; 2.4015x over previous
"""Causal self-attention with RoPE on 8 Trainium2 NeuronCores.

Full inputs: x [4, 2048, 1024], W_attn [1024, 3072], W_proj [1024, 1024] (f32).
Sharding: core = b*2 + hg  (b in 0..3 batches, hg in 0..1 head-groups of 8 heads).
Each core computes qkv for its 8 heads, attention, and a partial output
projection (row-parallel c_proj); host sums the two partials per batch.

All matmuls run in float32r (fp32 with 11-bit mantissa, full PE rate at N>=256).
Matmul operands are pre-rounded (host inputs) or produced by DVE/ACT ops that
write float32r directly.
"""

import sys

sys.path.insert(0, "/opt/trn_rl_repo")

import numpy as np

import concourse.bass as bass  # noqa: F401
import concourse.mybir as mybir
import concourse.tile as tile
from concourse import bacc
from concourse.bass_utils import run_bass_kernel_spmd

F32 = mybir.dt.float32
F32R = mybir.dt.float32r
AF = mybir.ActivationFunctionType
OP = mybir.AluOpType

B, T, C = 4, 2048, 1024
H, D = 16, 64
HPC = 8            # heads per core
CO_QKV = 3 * HPC * D   # 1536 qkv columns per core
NEG = -30000.0     # additive mask; exp(S + NEG) == 0 exactly on ACT

TC = 256           # t-chunk width in phase A
N_TC = T // TC     # 8
N_KO = C // 128    # 8 contraction chunks
N_CT = 2 * HPC * D // 128   # 8 q+k column tiles (4 q, 4 k)
N_TT = T // 128    # 16 t tiles
N_IC = 4           # i-chunks of 512 queries
IC = 512


def round_fp32r(x):
    b = np.ascontiguousarray(x, dtype=np.float32).view(np.uint32)
    r = ((b.astype(np.uint64) + 0x800) & 0xFFFFF000).astype(np.uint32)
    return r.view(np.float32).reshape(x.shape)


def _rope_tables():
    """cosT/sinN [128, T] f32: row p holds freq for d = p % 64; sinN has the
    rotate-half sign folded in (rows d<32 negative)."""
    inv_freq = (
        np.float32(1.0)
        / np.float32(10000.0) ** (np.arange(0, D, 2, dtype=np.float32) / np.float32(D))
    ).astype(np.float32)
    t = np.arange(T, dtype=np.float32)
    freqs = (t[:, None] * inv_freq[None, :]).astype(np.float32)  # [T, 32]
    emb = np.concatenate([freqs, freqs], axis=1)  # [T, 64]
    cos = np.cos(emb).astype(np.float32)
    sin = np.sin(emb).astype(np.float32)
    sinN = np.concatenate([-sin[:, :32], sin[:, 32:]], axis=1)
    cosT = np.tile(cos.T, (2, 1))   # [128, T]
    sinNT = np.tile(sinN.T, (2, 1))
    return np.ascontiguousarray(cosT), np.ascontiguousarray(sinNT)


def _mask_table():
    """mask [128, 896]: mask[j, c] = 0 if j <= c - 384 else NEG."""
    j = np.arange(128)[:, None]
    c = np.arange(896)[None, :]
    return np.where(j <= c - 384, 0.0, NEG).astype(np.float32)


def _build():
    nc = bacc.Bacc(None, target_bir_lowering=False, debug=False)

    xT = nc.dram_tensor("xT", [C, T], F32R, kind="ExternalInput")
    wqkv = nc.dram_tensor("wqkv", [C, CO_QKV], F32R, kind="ExternalInput")
    wproj = nc.dram_tensor("wproj", [HPC * D, C], F32R, kind="ExternalInput")
    cosT_d = nc.dram_tensor("cosT", [128, T], F32, kind="ExternalInput")
    sinN_d = nc.dram_tensor("sinN", [128, T], F32, kind="ExternalInput")
    mask_d = nc.dram_tensor("mask", [128, 896], F32, kind="ExternalInput")
    ones_d = nc.dram_tensor("ones", [128, 64], F32R, kind="ExternalInput")
    out_d = nc.dram_tensor("out", [T, C], F32, kind="ExternalOutput")

    xT_r = xT.rearrange("(ko p) t -> p ko t", p=128)
    wqkv_r = wqkv.rearrange("(ko p) c -> p ko c", p=128)
    wproj_r = wproj.rearrange("(ko p) c -> p ko c", p=128)

    with tile.TileContext(nc) as tc:
        with (
            tc.tile_pool(name="resident", bufs=1) as res,
            tc.tile_pool(name="qkv", bufs=1) as qkv_pool,
        ):
            # ---- resident tables + outputs of phase A ----
            cos_sb = res.tile([128, T], F32)
            sinN_sb = res.tile([128, T], F32)
            mask_sb = res.tile([128, 896], F32)
            ones_sb = res.tile([128, 64], F32R)
            nc.sync.dma_start(cos_sb[:], cosT_d[:])
            nc.sync.dma_start(sinN_sb[:], sinN_d[:])
            nc.sync.dma_start(mask_sb[:], mask_d[:])
            nc.sync.dma_start(ones_sb[:], ones_d[:])

            # q^T / k^T: [p = d within head-pair, hp, t]
            qT = qkv_pool.tile([128, HPC // 2, T], F32R)
            kT = qkv_pool.tile([128, HPC // 2, T], F32R)
            # v: [p = t%128, t//128, head, 65] with ones column at d=64
            v_sb = qkv_pool.tile([128, N_TT, HPC, D + 1], F32R)
            # ---------------- Phase A: QKV + RoPE ----------------
            with (
                tc.tile_pool(name="w", bufs=1) as wp,
                tc.tile_pool(name="xt", bufs=2) as xtp,
                tc.tile_pool(name="rope", bufs=3) as ropep,
                tc.tile_pool(name="ps_qk", bufs=3, space="PSUM") as ps_qk,
                tc.tile_pool(name="ps_v", bufs=2, space="PSUM") as ps_v,
            ):
                w_sb = wp.tile([128, N_KO, CO_QKV], F32R)
                nc.sync.dma_start(w_sb[:], wqkv_r[:])

                # ones columns of v (written once)
                for to in range(N_TT):
                    nc.vector.tensor_copy(v_sb[:, to, :, D], ones_sb[:, 0:HPC])

                for tc_i in range(N_TC):
                    ts_ = slice(tc_i * TC, (tc_i + 1) * TC)
                    xt_sb = xtp.tile([128, N_KO, TC], F32R)
                    nc.sync.dma_start(xt_sb[:], xT_r[:, :, ts_])

                    # q and k column tiles (ct 0-3 -> q head-pair, 4-7 -> k)
                    for ct in range(N_CT):
                        psum = ps_qk.tile([128, TC], F32)
                        for ko in range(N_KO):
                            nc.tensor.matmul(
                                psum[:],
                                w_sb[:, ko, ct * 128 : (ct + 1) * 128],
                                xt_sb[:, ko, :],
                                start=(ko == 0),
                                stop=(ko == N_KO - 1),
                            )
                        hp = ct % 4
                        dest = (qT if ct < 4 else kT)[:, hp, ts_]
                        # RoPE: dest = psum * cos + shift(psum) * sinN
                        rot = ropep.tile([128, TC], F32)
                        for blk in range(4):
                            src = (blk ^ 1) * 32
                            nc.vector.tensor_copy(
                                rot[blk * 32 : blk * 32 + 32, :],
                                psum[src : src + 32, :],
                            )
                        nc.vector.tensor_tensor(rot[:], rot[:], sinN_sb[:, ts_], OP.mult)
                        nc.vector.tensor_tensor(dest, psum[:], cos_sb[:, ts_], OP.mult)
                        nc.vector.tensor_tensor(dest, dest, rot[:], OP.add)

                    # v tiles: two 128-row t sub-tiles per chunk
                    for sub in range(TC // 128):
                        to = tc_i * (TC // 128) + sub
                        psv = ps_v.tile([128, HPC * D], F32)
                        for ko in range(N_KO):
                            nc.tensor.matmul(
                                psv[:],
                                xt_sb[:, ko, sub * 128 : sub * 128 + 128],
                                w_sb[:, ko, 2 * HPC * D : 3 * HPC * D],
                                start=(ko == 0),
                                stop=(ko == N_KO - 1),
                            )
                        nc.vector.tensor_copy(
                            v_sb[:, to, :, 0:D],
                            psv[:].rearrange("p (h d) -> p h d", d=D),
                        )

            # ---------------- Phases B+C pool (opens after phase A frees W/xT) ----
            with tc.tile_pool(name="yt", bufs=1) as ytp:
                # y^T: same layout as qT
                yT = ytp.tile([128, HPC // 2, T], F32R)
                _phase_bc(nc, tc, qT, kT, v_sb, yT, mask_sb, ones_sb, wproj_r, out_d)

    nc.compile()
    return nc


def _phase_bc(nc, tc, qT, kT, v_sb, yT, mask_sb, ones_sb, wproj_r, out_d):
    if True:
        if True:
            # ---------------- Phase B: attention ----------------
            with (
                tc.tile_pool(name="exp", bufs=6) as expp,
                tc.tile_pool(name="smask", bufs=3) as smp,
                tc.tile_pool(name="fin", bufs=4) as finp,
                tc.tile_pool(name="ps_s", bufs=3, space="PSUM") as ps_s,
                tc.tile_pool(name="ps_y", bufs=2, space="PSUM") as ps_y,
                tc.tile_pool(name="ps_bc", bufs=2, space="PSUM") as ps_bc,
            ):
                for hp in range(HPC // 2):
                    for ic in range(N_IC):
                        is_ = slice(ic * IC, (ic + 1) * IC)
                        n_jt = (ic + 1) * 4
                        ypsum = [
                            ps_y.tile([D + 1, IC], F32, name=f"ypsum{_hl}", tag="ypsum")
                            for _hl in range(2)
                        ]
                        for jt in range(n_jt):
                            for hl in range(2):
                                pb = hl * 64
                                h = 2 * hp + hl
                                sps = ps_s.tile([128, IC], F32)
                                nc.tensor.matmul(
                                    sps[:],
                                    kT[pb : pb + 64, hp, jt * 128 : (jt + 1) * 128],
                                    qT[pb : pb + 64, hp, is_],
                                    start=True,
                                    stop=True,
                                )
                                expT = expp.tile([128, IC], F32R)
                                if jt >= ic * 4:
                                    off = 384 - (jt - ic * 4) * 128
                                    sm = smp.tile([128, IC], F32)
                                    nc.vector.tensor_tensor(
                                        sm[:], sps[:], mask_sb[:, off : off + IC], OP.add
                                    )
                                    nc.scalar.activation(expT[:], sm[:], AF.Exp)
                                else:
                                    nc.scalar.activation(expT[:], sps[:], AF.Exp)
                                nc.tensor.matmul(
                                    ypsum[hl][:],
                                    v_sb[:, jt, h, :],
                                    expT[:],
                                    start=(jt == 0),
                                    stop=(jt == n_jt - 1),
                                )
                        for hl in range(2):
                            pb = hl * 64
                            recip = finp.tile([1, IC], F32R)
                            with nc.allow_low_precision(reason="softmax recip f32r"):
                                nc.vector.reciprocal(recip[:], ypsum[hl][D : D + 1, :])
                            bc = ps_bc.tile([D, IC], F32)
                            nc.tensor.matmul(
                                bc[:], ones_sb[0:1, 0:D], recip[:], start=True, stop=True
                            )
                            bc_sb = finp.tile([D, IC], F32, name="bc_sb", tag="bc_sb")
                            nc.vector.tensor_copy(bc_sb[:], bc[:])
                            nc.vector.tensor_tensor(
                                yT[pb : pb + 64, hp, is_],
                                ypsum[hl][0:D, :],
                                bc_sb[:],
                                OP.mult,
                            )

            # ---------------- Phase C: output projection ----------------
            with (
                tc.tile_pool(name="wp2", bufs=1) as wp2,
                tc.tile_pool(name="ostage", bufs=4) as osp,
                tc.tile_pool(name="ps_o", bufs=4, space="PSUM") as ps_o,
            ):
                wp_sb = wp2.tile([128, 4, C], F32R)
                nc.sync.dma_start(wp_sb[:], wproj_r[:])
                for tt in range(N_TT):
                    for cc in range(C // 512):
                        po = ps_o.tile([128, 512], F32)
                        for ko in range(4):
                            nc.tensor.matmul(
                                po[:],
                                yT[:, ko, tt * 128 : (tt + 1) * 128],
                                wp_sb[:, ko, cc * 512 : (cc + 1) * 512],
                                start=(ko == 0),
                                stop=(ko == 3),
                            )
                        ost = osp.tile([128, 512], F32)
                        nc.vector.tensor_copy(ost[:], po[:])
                        nc.sync.dma_start(
                            out_d[tt * 128 : (tt + 1) * 128, cc * 512 : (cc + 1) * 512],
                            ost[:],
                        )


_NC = None
_STATE = None


def _get_nc():
    global _NC
    if _NC is None:
        _NC = _build()
    return _NC


def _get_state():
    """Build the bass module once and cache a persistent jitted executor plus
    device-resident static tables (rope/mask/ones are pure functions of the
    problem shape)."""
    global _STATE
    if _STATE is not None:
        return _STATE

    import jax
    import jax.numpy as jnp
    from jax.experimental.shard_map import shard_map
    from jax.sharding import Mesh, NamedSharding, PartitionSpec

    from concourse import bass2jax

    nc = _get_nc()
    bass2jax.install_neuronx_cc_hook()
    partition_name = nc.partition_id_tensor.name if nc.partition_id_tensor else None
    in_names, out_names, out_avals = [], [], []
    for alloc in nc.m.functions[0].allocations:
        if not isinstance(alloc, mybir.MemoryLocationSet):
            continue
        name = alloc.memorylocations[0].name
        if alloc.kind == "ExternalInput":
            if name != partition_name:
                in_names.append(name)
        elif alloc.kind == "ExternalOutput":
            out_names.append(name)
            out_avals.append(
                jax.core.ShapedArray(tuple(alloc.tensor_shape), mybir.dt.np(alloc.dtype))
            )
    n_params, n_outs = len(in_names), len(out_avals)
    all_names = list(in_names) + out_names
    if partition_name:
        all_names.append(partition_name)

    def _body(*args):
        operands = list(args)
        if partition_name:
            operands.append(bass2jax.partition_id_tensor())
        outs = bass2jax._bass_exec_p.bind(
            *operands,
            out_avals=tuple(out_avals),
            in_names=tuple(all_names),
            out_names=tuple(out_names),
            lowering_input_output_aliases=(),
            sim_require_finite=True,
            sim_require_nnan=True,
            nc=nc,
        )
        return tuple(outs)

    devices = jax.devices()[:8]
    mesh = Mesh(np.asarray(devices), ("core",))
    shd = NamedSharding(mesh, PartitionSpec("core"))
    donate = tuple(range(n_params, n_params + n_outs))
    sharded = jax.jit(
        shard_map(
            _body,
            mesh=mesh,
            in_specs=(PartitionSpec("core"),) * (n_params + n_outs),
            out_specs=(PartitionSpec("core"),) * n_outs,
            check_rep=False,
        ),
        donate_argnums=donate,
        keep_unused=True,
    )
    zeros_fn = jax.jit(
        lambda: tuple(
            jnp.zeros((8 * av.shape[0],) + av.shape[1:], av.dtype) for av in out_avals
        ),
        out_shardings=(shd,) * n_outs,
    )

    cosT, sinN = _rope_tables()
    mask = _mask_table()
    ones = np.ones((128, 64), dtype=np.float32)
    statics = {
        "cosT": jax.device_put(np.tile(cosT, (8, 1)), shd),
        "sinN": jax.device_put(np.tile(sinN, (8, 1)), shd),
        "mask": jax.device_put(np.tile(mask, (8, 1)), shd),
        "ones": jax.device_put(np.tile(ones, (8, 1)), shd),
    }
    jax.block_until_ready(list(statics.values()))

    # On-device input dedup (x shared by core pairs, W by head-groups) and
    # output pair-reduction, as separate XLA modules.
    PAIRS = [[0, 1], [2, 3], [4, 5], [6, 7]]
    QUADS = [[0, 2, 4, 6], [1, 3, 5, 7]]

    def _pre(xs, wq, wp):
        xg = jax.lax.all_gather(xs, "core", axis_index_groups=PAIRS, axis=0, tiled=True)
        wqg = jax.lax.all_gather(wq, "core", axis_index_groups=QUADS, axis=0, tiled=True)
        wpg = jax.lax.all_gather(wp, "core", axis_index_groups=QUADS, axis=0, tiled=True)
        return xg.T, wqg, wpg

    pre_fn = jax.jit(
        shard_map(
            _pre,
            mesh=mesh,
            in_specs=(PartitionSpec("core"),) * 3,
            out_specs=(PartitionSpec("core"),) * 3,
        )
    )

    def _post(o):
        other = jax.lax.ppermute(
            o, "core", [(0, 1), (1, 0), (2, 3), (3, 2), (4, 5), (5, 4), (6, 7), (7, 6)]
        )
        s = o + other
        idx = jax.lax.axis_index("core")
        return jax.lax.dynamic_slice(s, ((idx % 2) * (T // 2), 0), (T // 2, C))

    post_fn = jax.jit(
        shard_map(
            _post,
            mesh=mesh,
            in_specs=(PartitionSpec("core"),),
            out_specs=PartitionSpec("core"),
        )
    )

    _STATE = dict(
        jax=jax,
        nc=nc,
        in_names=in_names,
        out_names=out_names,
        n_outs=n_outs,
        sharded=sharded,
        zeros_fn=zeros_fn,
        shd=shd,
        statics=statics,
        pre_fn=pre_fn,
        post_fn=post_fn,
    )
    return _STATE


def _prep_compact(x, W_attn, W_proj):
    """Compact (dedup'd) host inputs for the on-device gather pre-module.

    xs:  [8*1024, 1024] — core c gets rows of x[c//2] half c%2 (no transpose,
         no duplication; the device gathers pairs and transposes).
    wq:  [8*256, 1536]  — core c gets a quarter of wqkv_{c%2}.
    wp:  [8*128, 1024]  — core c gets a quarter of wproj_{c%2}.
    The hardware rounds fp32r matmul operands itself, so no host rounding.
    """
    scale = np.float32(1.0 / np.sqrt(D))
    xs = np.ascontiguousarray(x.reshape(8 * (T // 2), C))
    wqkv_hg = []
    wproj_hg = []
    for hg in range(2):
        cs = slice(hg * HPC * D, (hg + 1) * HPC * D)
        wq = W_attn[:, 0 * C:][:, cs] * scale
        wk = W_attn[:, 1 * C:][:, cs]
        wv = W_attn[:, 2 * C:][:, cs]
        wqkv_hg.append(np.concatenate([wq, wk, wv], axis=1))
        wproj_hg.append(W_proj[cs, :])
    wqcat = np.empty((8 * 256, CO_QKV), dtype=np.float32)
    wpcat = np.empty((8 * 128, C), dtype=np.float32)
    for c in range(8):
        q = c // 2
        wqcat[c * 256 : (c + 1) * 256] = wqkv_hg[c % 2][q * 256 : (q + 1) * 256]
        wpcat[c * 128 : (c + 1) * 128] = wproj_hg[c % 2][q * 128 : (q + 1) * 128]
    return xs, wqcat, wpcat


def _run_gathered(st, x, W_attn, W_proj):
    jax = st["jax"]
    xs, wqcat, wpcat = _prep_compact(x, W_attn, W_proj)
    d_xs = jax.device_put(xs, st["shd"])
    d_wq = jax.device_put(wqcat, st["shd"])
    d_wp = jax.device_put(wpcat, st["shd"])
    xt, wqg, wpg = st["pre_fn"](d_xs, d_wq, d_wp)
    dyn = {"xT": xt, "wqkv": wqg, "wproj": wpg}
    args = [dyn[nm] if nm in dyn else st["statics"][nm] for nm in st["in_names"]]
    zeros = st["zeros_fn"]()
    outs = st["sharded"](*args, *zeros)
    po = st["post_fn"](outs[0])
    r = np.asarray(po).reshape(B, T, C)
    return np.ascontiguousarray(r)


def _run_plain(st, x, W_attn, W_proj):
    """Fallback: duplicated per-core inputs, host-side pair sum."""
    jax = st["jax"]
    scale = np.float32(1.0 / np.sqrt(D))
    xcat = np.empty((8 * C, T), dtype=np.float32)
    for b in range(B):
        xt = np.ascontiguousarray(x[b].T)
        xcat[(2 * b) * C : (2 * b + 1) * C] = xt
        xcat[(2 * b + 1) * C : (2 * b + 2) * C] = xt
    wqkvcat = np.empty((8 * C, CO_QKV), dtype=np.float32)
    wprojcat = np.empty((8 * HPC * D, C), dtype=np.float32)
    for hg in range(2):
        cs = slice(hg * HPC * D, (hg + 1) * HPC * D)
        wq = W_attn[:, 0 * C:][:, cs] * scale
        wk = W_attn[:, 1 * C:][:, cs]
        wv = W_attn[:, 2 * C:][:, cs]
        wqkv = np.concatenate([wq, wk, wv], axis=1)
        wproj = W_proj[cs, :]
        for b in range(B):
            core = 2 * b + hg
            wqkvcat[core * C : (core + 1) * C] = wqkv
            wprojcat[core * HPC * D : (core + 1) * HPC * D] = wproj
    dyn = {"xT": xcat, "wqkv": wqkvcat, "wproj": wprojcat}
    args = [
        jax.device_put(dyn[nm], st["shd"]) if nm in dyn else st["statics"][nm]
        for nm in st["in_names"]
    ]
    zeros = st["zeros_fn"]()
    outs = st["sharded"](*args, *zeros)
    r = np.asarray(outs[0]).reshape(8, T, C)
    out = np.empty((B, T, C), dtype=np.float32)
    for b in range(B):
        np.add(r[2 * b], r[2 * b + 1], out=out[b])
    return out


def kernel(x, W_attn, W_proj):
    x = np.asarray(x, dtype=np.float32)
    W_attn = np.asarray(W_attn, dtype=np.float32)
    W_proj = np.asarray(W_proj, dtype=np.float32)

    st = _get_state()
    try:
        return _run_gathered(st, x, W_attn, W_proj)
    except Exception:
        return _run_plain(st, x, W_attn, W_proj)
